# revision 18
# baseline (speedup 1.0000x reference)
"""GCNAggregator Trainium2 Bass kernel.

out[i] = (sum_{e: seg[e]==i} features[neighbor_idx[e]] + features[i]) / (deg_i + 1)

Strategy (8 NeuronCores, SPMD):
  - Nodes are sharded into 8 contiguous, edge-balanced ranges (<=6272 nodes
    each). Since segment_ids is sorted, each core's incident edges are a
    contiguous range of the edge list. The self-loop is folded in as one
    extra edge per node, so the whole aggregation is a single segment-sum.
  - Per core, nodes are packed greedily into "slots" of <=128 consecutive
    nodes, capped by per-slot edge counts so every slot is nearly full of
    edges. Slot tile counts are maxed over the 8 cores so the compiled
    program is identical on every core (SPMD) at ~2% gather padding.
  - Features are stored as a bf16 table (512B/row). Each slot's edges are
    gathered from HBM with gpsimd.dma_gather, then segment-summed on the
    tensor engine via one 256-wide one-hot matmul per 128-edge tile:
        psum[128 nodes, 256] += onehot[128 edges, 128 nodes]^T @ G[128 edges, 256]
    The one-hot is exact in bf16 and PSUM accumulates in fp32 (end-to-end
    rel err ~3e-3, well inside the 2e-2 gate) at 1 cycle/row matmul
    throughput. The one-hot is built on device from per-edge relative
    segment ids (is_equal vs an iota row, all-bf16 for 2x DVE throughput).
  - dma_gather indices are int16 (max 32767) but the table has 50000 rows,
    so each slot's edges are split into a low class (row < 32768) and a
    high class (row >= 32768, gathered from an offset view of the table).
  - The SWDGE descriptor ring caps dma_gather at 1024 descriptors per call
    (larger rings wedge the device), and each call costs ~1us fixed
    descriptor-generation time on the Pool engine. To stay off that
    bottleneck, the L- and H-class rows of ALL slots form two contiguous
    streams gathered with full-ring 1024-row calls (independent of slot
    boundaries) into circular SBUF tile rings; slot matmuls read their
    tiles from the rings, with ordering enforced by the tile framework's
    subtile dependency tracking.
  - Finalize per slot: out = psum * 1/(deg+1) (bf16), DMA out.
  - Engine budget per core (TimelineSim): DMA ~325us (98% busy, the
    bottleneck: 214k gather descriptors x 512B at 360GB/s aggregate),
    Pool ~281us, PE ~181us, DVE ~179us.

The host only computes integer index metadata (shard boundaries, per-slot
class-split index streams, relative segment ids, degrees); all floating
point work (gather, segment sum, normalize) runs on device.
"""

import sys

import numpy as np

try:
    import concourse  # noqa: F401
except ImportError:  # pragma: no cover
    sys.path.insert(0, "/opt/trn_rl_repo")

from contextlib import ExitStack

import concourse.mybir as mybir
from concourse import bacc, bass_utils, tile

N_NODES = 50000
N_EDGES = 1_600_000
D = 256
N_CORES = 8
NPC = 6272          # node slots per core (= GROUPS * 128)
GROUPS = 49
SPLIT = 32768       # int16 gather-index window

_PROGRAM_CACHE: dict = {}
LAST_NC = None  # exposed for test harness introspection (TimelineSim)

import os as _os

DMA_SCRATCH = int(_os.environ.get("DMA_SCRATCH", "16384"))
MAX_GATHER = DMA_SCRATCH // 16  # SWDGE descriptor ring capacity per call
GIDX16 = _os.environ.get("GIDX16", "0") == "1"  # un-replicated 16-row gidx
GIDX_P = 16 if GIDX16 else 128
# replicate the 16-row wrapped gather-index stream to 128 partitions on the
# tensor engine (f32 one-hot matmul, exact for idx values < 2^24) instead of
# shipping it 8x-replicated over the wire
GIDXPE = _os.environ.get("GIDXPE", "0") == "1"


RING_L = 96   # L-stream SBUF ring, in 128-row tiles (multiple of 8)
RING_H = 56   # H-stream ring


def _build_program(t_l_arr: tuple, t_h_arr: tuple):
    """Build + compile the (uniform-across-cores, SPMD) per-core program.

    t_l_arr/t_h_arr: per-group tile counts (max over the 8 cores), so the
    program structure is identical on every core while padding stays low.

    The L- and H-class gather rows of all slots form two contiguous streams;
    each stream is gathered with full MAX_GATHER-descriptor dma_gather calls
    (independent of slot boundaries) into a circular SBUF ring of 128-row
    tiles. Slot matmuls read their tiles from the rings; the tile
    framework's subtile dependency tracking orders ring reuse.
    """
    n_slots = len(t_l_arr)
    tiles_g = [t_l_arr[g] + t_h_arr[g] for g in range(n_slots)]
    nt_tot = sum(tiles_g)
    rows_tot = nt_tot * 128
    ni16 = rows_tot // 16  # gidx columns (wrapped-16 int16 layout)
    nt_l = sum(t_l_arr)
    nt_h = sum(t_h_arr)
    rows_l = nt_l * 128
    cum_lt = np.concatenate([[0], np.cumsum(t_l_arr)]).astype(int)
    cum_ht = np.concatenate([[0], np.cumsum(t_h_arr)]).astype(int)

    nc = bacc.Bacc(
        "TRN2", target_bir_lowering=False, debug=False, num_devices=N_CORES,
        dynamic_dma_scratch_size=DMA_SCRATCH,
    )

    feat_d = nc.dram_tensor(
        "featb", (N_NODES, D), mybir.dt.bfloat16, kind="ExternalInput"
    ).ap()
    if GIDXPE:
        gidx_d = nc.dram_tensor(
            "gidxf", (16, ni16), mybir.dt.float32, kind="ExternalInput"
        ).ap()
        rep_d = nc.dram_tensor(
            "repmat", (16, 128), mybir.dt.float32, kind="ExternalInput"
        ).ap()
    else:
        gidx_d = nc.dram_tensor(
            "gidx", (GIDX_P, ni16), mybir.dt.int16, kind="ExternalInput"
        ).ap()
    srel_d = nc.dram_tensor(
        "srel", (128, nt_tot), mybir.dt.bfloat16, kind="ExternalInput"
    ).ap()
    cnt1_d = nc.dram_tensor(
        "cnt1", (128, n_slots), mybir.dt.float32, kind="ExternalInput"
    ).ap()
    out_d = nc.dram_tensor(
        "out", (n_slots * 128, D), mybir.dt.bfloat16, kind="ExternalOutput"
    ).ap()

    feat_lo = feat_d[0:SPLIT, :]
    feat_hi = feat_d[SPLIT:N_NODES, :]

    with tile.TileContext(nc) as tc:
        with ExitStack() as ctx:
            import os

            ob = int(os.environ.get("OH_BUFS", "4"))
            fb = int(os.environ.get("FIN_BUFS", "3"))
            pb = int(os.environ.get("PSUM_BUFS", "4"))
            const_pool = ctx.enter_context(tc.tile_pool(name="const", bufs=1))
            oh_pool = ctx.enter_context(tc.tile_pool(name="oh", bufs=ob))
            fin_pool = ctx.enter_context(tc.tile_pool(name="fin", bufs=fb))
            psum_pool = ctx.enter_context(
                tc.tile_pool(name="psum", bufs=pb, space="PSUM")
            )

            # persistent metadata in SBUF (gidx loaded in chunks so early
            # gathers don't wait on the full index transfer)
            if GIDXPE:
                gidx_sb = const_pool.tile([128, ni16], mybir.dt.int16)
                gidxf_sb = const_pool.tile([16, ni16], mybir.dt.float32)
                rep_sb = const_pool.tile([16, 128], mybir.dt.float32)
                nc.sync.dma_start(rep_sb[:], rep_d[:])
                n_ld = 8
                ld_bounds = [ni16 * i // n_ld for i in range(n_ld + 1)]
                for a, b in zip(ld_bounds[:-1], ld_bounds[1:]):
                    if b > a:
                        nc.sync.dma_start(gidxf_sb[:, a:b], gidx_d[:, a:b])
                rep_pool = ctx.enter_context(
                    tc.tile_pool(name="rpsum", bufs=2, space="PSUM")
                )
                rep_bounds = [0]
                while rep_bounds[-1] < ni16:
                    a = rep_bounds[-1]
                    step = 128 if a < 1024 else 512
                    rep_bounds.append(min(a + step, ni16))
                for a, b in zip(rep_bounds[:-1], rep_bounds[1:]):
                    rp = rep_pool.tile([128, 512], mybir.dt.float32, tag="rp")
                    nc.tensor.matmul(
                        rp[:, : b - a], rep_sb[:], gidxf_sb[:, a:b],
                        start=True, stop=True,
                    )
                    nc.vector.tensor_copy(gidx_sb[:, a:b], rp[:, : b - a])
            else:
                gidx_sb = const_pool.tile([GIDX_P, ni16], mybir.dt.int16)
                n_ld = 8
                ld_bounds = [ni16 * i // n_ld for i in range(n_ld + 1)]
                for a, b in zip(ld_bounds[:-1], ld_bounds[1:]):
                    if b > a:
                        nc.sync.dma_start(gidx_sb[:, a:b], gidx_d[:, a:b])
            # srel rides the wire as bf16 (values are small integers, exact)
            # and is widened on device: tensor_scalar's scalar operand must
            # be f32.
            srel_bf = const_pool.tile([128, nt_tot], mybir.dt.bfloat16)
            nc.sync.dma_start(srel_bf[:], srel_d[:])
            srel_sb = const_pool.tile([128, nt_tot], mybir.dt.float32)
            nc.vector.tensor_copy(srel_sb[:], srel_bf[:])
            cnt1_sb = const_pool.tile([128, n_slots], mybir.dt.float32)
            nc.sync.dma_start(cnt1_sb[:], cnt1_d[:])

            iota_i = const_pool.tile([128, 128], mybir.dt.int32)
            nc.gpsimd.iota(iota_i[:], pattern=[[1, 128]], base=0, channel_multiplier=0)
            iota_f = const_pool.tile([128, 128], mybir.dt.bfloat16)
            nc.vector.tensor_copy(iota_f[:], iota_i[:])

            ring_l = const_pool.tile([128, RING_L, D], mybir.dt.bfloat16)
            ring_h = const_pool.tile([128, RING_H, D], mybir.dt.bfloat16)

            def emit_call(ring, ring_sz, src, row0, rows_end, col0):
                """One full-ring-slice gather call of the given stream."""
                k = min(MAX_GATHER, rows_end - row0)
                s0 = (row0 // 128) % ring_sz
                nc.gpsimd.dma_gather(
                    ring[:, s0 : s0 + k // 128, :], src,
                    gidx_sb[:, col0 + row0 // 16 : col0 + (row0 + k) // 16],
                    num_idxs=k, num_idxs_reg=k,
                    elem_size=D, elem_step=D,
                )
                return row0 + k

            done_l = 0  # stream rows gathered so far
            done_h = 0
            for g in range(n_slots):
                t_l = t_l_arr[g]
                n_tiles = tiles_g[g]
                while done_l < cum_lt[g + 1] * 128:
                    done_l = emit_call(ring_l, RING_L, feat_lo, done_l,
                                       nt_l * 128, 0)
                while done_h < cum_ht[g + 1] * 128:
                    done_h = emit_call(ring_h, RING_H, feat_hi, done_h,
                                       nt_h * 128, rows_l // 16)

                psum = psum_pool.tile([128, D], mybir.dt.float32, tag="ps")
                for t in range(n_tiles):
                    if t < t_l:
                        m = int(cum_lt[g]) + t
                        gt_tile = ring_l[:, m % RING_L, :]
                    else:
                        m = nt_l + int(cum_ht[g]) + (t - t_l)
                        gt_tile = ring_h[:, (m - nt_l) % RING_H, :]
                    oh = oh_pool.tile([128, 128], mybir.dt.bfloat16, tag="oh")
                    nc.vector.tensor_scalar(
                        oh[:], iota_f[:], srel_sb[:, m : m + 1], None,
                        op0=mybir.AluOpType.is_equal,
                    )
                    nc.tensor.matmul(
                        psum[:], oh[:], gt_tile,
                        start=(t == 0), stop=(t == n_tiles - 1),
                    )

                rec = fin_pool.tile([128, 1], mybir.dt.float32, tag="rec")
                nc.vector.reciprocal(rec[:], cnt1_sb[:, g : g + 1])
                o_sb = fin_pool.tile([128, D], mybir.dt.bfloat16, tag="o")
                nc.vector.tensor_scalar_mul(o_sb[:], psum[:], rec[:])
                nc.sync.dma_start(out_d[g * 128 : (g + 1) * 128, :], o_sb[:])

    nc.compile()
    return nc


def _pack_slots(cum_l, cum_h, n_nodes, cap_l, cap_h):
    """Greedy variable-width node slots: each slot takes consecutive nodes
    (<=128) while its L/H edge counts stay under the caps. Returns a list of
    (base, width, nL, nH)."""
    slots = []
    i = 0
    while i < n_nodes:
        jmax = min(i + 128, n_nodes)
        jl = int(np.searchsorted(cum_l, cum_l[i] + cap_l * 128, side="right")) - 1
        jh = int(np.searchsorted(cum_h, cum_h[i] + cap_h * 128, side="right")) - 1
        j = max(min(jmax, jl, jh), i + 1)
        slots.append(
            (i, j - i, int(cum_l[j] - cum_l[i]), int(cum_h[j] - cum_h[i]))
        )
        i = j
    return slots


def _preprocess(features, neighbor_idx, segment_ids):
    """Host-side shard/index metadata construction (integers only)."""
    feat = np.ascontiguousarray(np.asarray(features, dtype=np.float32))
    seg = np.asarray(segment_ids).astype(np.int64)
    nid = np.asarray(neighbor_idx).astype(np.int64)
    n_edges = seg.shape[0]

    bf16 = mybir.dt.np(mybir.dt.bfloat16)
    featb = feat.astype(bf16)

    deg = np.bincount(seg, minlength=N_NODES)

    # edge-balanced core node boundaries (spans capped at NPC node slots)
    bounds = [0]
    for c in range(1, N_CORES):
        n = int(seg[min(c * n_edges // N_CORES, n_edges - 1)])
        n = min(n, bounds[-1] + NPC)
        n = max(n, N_NODES - (N_CORES - c) * NPC, bounds[-1])
        bounds.append(n)
    bounds.append(N_NODES)

    # per-core edge slices (self-loop folded in as one extra edge per node)
    # and per-node class-split prefix sums
    per_core = []
    for c in range(N_CORES):
        lo, hi = np.searchsorted(seg, [bounds[c], bounds[c + 1]])
        nn = bounds[c + 1] - bounds[c]
        s = np.concatenate([seg[lo:hi] - bounds[c], np.arange(nn)])
        x = np.concatenate([nid[lo:hi], np.arange(bounds[c], bounds[c + 1])])
        order = np.argsort(s, kind="stable")
        s = s[order]
        x = x[order]
        is_l = x < SPLIT
        cnt_l = np.bincount(s[is_l], minlength=nn)
        cnt_h = np.bincount(s[~is_l], minlength=nn)
        cum_l = np.concatenate([[0], np.cumsum(cnt_l)])
        cum_h = np.concatenate([[0], np.cumsum(cnt_h)])
        per_core.append((s, x, nn, cum_l, cum_h))

    # choose caps minimizing the max of the modeled DMA and Pool-engine
    # (SWDGE descriptor-gen) times: gather descriptors cost ~1.42ns each on
    # the shared DMA engines, while each dma_gather call costs ~1us fixed on
    # the Pool engine with at most MAX_GATHER descriptors per call.
    best = None
    for cap_l in range(8, 27):
        for cap_h in range(4, 15):
            all_slots = [
                _pack_slots(pc[3], pc[4], pc[2], cap_l, cap_h) for pc in per_core
            ]
            n_slots = max(len(sl) for sl in all_slots)
            tl = np.zeros(n_slots, np.int64)
            th = np.zeros(n_slots, np.int64)
            for sl in all_slots:
                for g, (_, _, nl, nh) in enumerate(sl):
                    tl[g] = max(tl[g], -(-nl // 128))
                    th[g] = max(th[g], -(-nh // 128))
            rows = 128 * int(tl.sum() + th.sum())
            calls = -(-128 * int(tl.sum()) // MAX_GATHER) + -(
                -128 * int(th.sum()) // MAX_GATHER
            )
            dma_ns = rows * 1.4225 + (13000 if GIDX16 else 22000)
            pool_ns = calls * 994 + rows * 0.34 + 1300
            score = max(dma_ns, pool_ns)
            if best is None or score < best[0]:
                best = (score, tuple(int(v) for v in tl), tuple(int(v) for v in th), all_slots)
    _, t_l_arr, t_h_arr, all_slots = best
    # a slot with zero tiles would leave its PSUM accumulator unwritten
    t_l_arr = tuple(
        max(tl, 1) if tl + th == 0 else tl for tl, th in zip(t_l_arr, t_h_arr)
    )
    n_slots = len(t_l_arr)

    nt_tot = sum(t_l_arr) + sum(t_h_arr)
    nt_l = sum(t_l_arr)
    rows_l = nt_l * 128
    cum_lt = np.concatenate([[0], np.cumsum(t_l_arr)]).astype(int)
    cum_ht = np.concatenate([[0], np.cumsum(t_h_arr)]).astype(int)

    in_maps = []
    slot_maps = []
    for c in range(N_CORES):
        s, x, nn, _, _ = per_core[c]
        slots = all_slots[c]
        gidx_all = np.zeros(nt_tot * 128, np.int16)
        srel_all = np.full((nt_tot, 128), -1.0, np.float32)
        cnt1 = np.ones((128, n_slots), np.float32)
        node_bnds = [sl[0] for sl in slots] + [nn]
        edge_bnds = np.searchsorted(s, node_bnds)
        for g, (base_n, width, _, _) in enumerate(slots):
            t_l, t_h = t_l_arr[g], t_h_arr[g]
            kl, kh = t_l * 128, t_h * 128
            a, b = edge_bnds[g], edge_bnds[g + 1]
            sg = s[a:b]
            xg = x[a:b]
            m = xg < SPLIT
            xl = xg[m]
            xh = xg[~m] - SPLIT
            sl_ = sg[m] - base_n
            sh_ = sg[~m] - base_n
            # sort each run by source row: the one-hot matmul is order-
            # invariant within a slot, and address-sorted gather descriptors
            # get HBM row-buffer locality (duplicates become adjacent)
            ol = np.argsort(xl, kind="stable")
            xl, sl_ = xl[ol], sl_[ol]
            oh_ = np.argsort(xh, kind="stable")
            xh, sh_ = xh[oh_], sh_[oh_]
            base_l = int(cum_lt[g]) * 128
            base_h = rows_l + int(cum_ht[g]) * 128
            gidx_all[base_l : base_l + len(xl)] = xl.astype(np.int16)
            gidx_all[base_h : base_h + len(xh)] = xh.astype(np.int16)
            srl = np.full(kl, -1.0, np.float32)
            srl[: len(sl_)] = sl_
            srel_all[cum_lt[g] : cum_lt[g] + t_l] = srl.reshape(t_l, 128)
            srh = np.full(kh, -1.0, np.float32)
            srh[: len(sh_)] = sh_
            srel_all[nt_l + cum_ht[g] : nt_l + cum_ht[g] + t_h] = (
                srh.reshape(t_h, 128)
            )
            abs_base = bounds[c] + base_n
            cnt1[:width, g] = 1.0 + deg[abs_base : abs_base + width]

        gidx_w = gidx_all.reshape(-1, 16).T
        if GIDXPE:
            gidx_w = np.ascontiguousarray(gidx_w.astype(np.float32))
        else:
            if not GIDX16:
                gidx_w = np.tile(gidx_w, (8, 1))
            gidx_w = np.ascontiguousarray(gidx_w)
        srel_mat = np.ascontiguousarray(srel_all.T).astype(bf16)
        imap = {
            "featb": featb,
            "srel": srel_mat,
            "cnt1": cnt1,
        }
        if GIDXPE:
            imap["gidxf"] = gidx_w
            imap["repmat"] = np.ascontiguousarray(
                (np.arange(128)[None, :] % 16 == np.arange(16)[:, None])
                .astype(np.float32)
            )
        else:
            imap["gidx"] = gidx_w
        in_maps.append(imap)
        slot_maps.append(
            [(bounds[c] + sl[0], sl[1]) for sl in slots]
        )
    return t_l_arr, t_h_arr, in_maps, slot_maps


def kernel(features, neighbor_idx, segment_ids):
    global LAST_NC
    t_l_arr, t_h_arr, in_maps, slot_maps = _preprocess(
        features, neighbor_idx, segment_ids
    )

    key = (t_l_arr, t_h_arr)
    if key not in _PROGRAM_CACHE:
        _PROGRAM_CACHE[key] = _build_program(t_l_arr, t_h_arr)
    nc = _PROGRAM_CACHE[key]
    LAST_NC = nc

    try:
        res = bass_utils.run_bass_kernel_spmd(
            nc, in_maps, core_ids=list(range(N_CORES))
        )
    except Exception:
        # transient axon/device hiccups (e.g. recovering from a prior wedge)
        # have been observed to clear after a short pause
        import time

        time.sleep(20)
        res = bass_utils.run_bass_kernel_spmd(
            nc, in_maps, core_ids=list(range(N_CORES))
        )

    out = np.empty((N_NODES, D), np.float32)
    for c in range(N_CORES):
        oc = res.results[c]["out"].astype(np.float32)
        for g, (abs_base, width) in enumerate(slot_maps[c]):
            out[abs_base : abs_base + width] = oc[g * 128 : g * 128 + width]
    return out


# revision 21
# speedup vs baseline: 1.0287x; 1.0287x over previous
"""GCNAggregator Trainium2 Bass kernel.

out[i] = (sum_{e: seg[e]==i} features[neighbor_idx[e]] + features[i]) / (deg_i + 1)

Strategy (8 NeuronCores, SPMD):
  - dma_gather indices are int16 (max 32767) but the table has 50000 rows,
    so every gathered row is classed L (row < 32768, gathered from the
    table base) or H (row >= 32768, gathered from an offset view). Each
    core is given a contiguous slice of the LOW dest nodes and a
    contiguous slice of the HIGH dest nodes, edge-balanced within each
    class, so per-core L/H gather totals match across cores to ~0.1%
    (one self-loop edge per node is folded in, and self rows are all-L on
    low nodes / all-H on high nodes).
  - Features ride as a bf16 table (512B/row -- the cost model's DMA
    sweet spot). Rel err ~3e-3 end-to-end, well inside the 2e-2 gate.
  - Per core, dest nodes are packed into 51 "slots" of <=128 consecutive
    nodes. The L rows of all slots form one dense stream in dest order
    (H likewise): no per-slot alignment padding. Slot boundaries are
    anchored to shared cumulative targets so every core's slot-g stream
    interval lands within a tile or two of the same place.
  - Streams are gathered with full 1024-descriptor dma_gather calls
    (the SWDGE ring caps at 1024 descriptors per call; bigger rings wedge
    the device) into circular SBUF rings of 128-row tiles, ~1us fixed
    Pool-engine cost per call. Gather calls ignore slot boundaries.
  - Segment-sum per slot on the tensor engine: for each stream tile
    overlapping the slot's interval on ANY core,
        psum[128 nodes, 256] += onehot[128 rows, 128 nodes]^T @ ring[tile]
    with the bf16 one-hot built on device by is_equal(iota, srel) from
    per-(tile,slot) relative dest ids; rows of a boundary tile that
    belong to the neighboring slot carry srel -1 there and are picked up
    by that slot's own matmul over the same tile. PSUM accumulates fp32.
  - Finalize per slot: out = psum * 1/(deg+1) (bf16), DMA out.
  - Engine budget per core (TimelineSim): DMA ~314us (98% busy, the
    bottleneck: 206k gather descriptors x 512B at 360GB/s aggregate),
    Pool ~273us, PE ~215us, DVE ~212us.

The host only computes integer index metadata (shard boundaries, stream
index layouts, relative segment ids, degrees); all floating point work
(gather, segment sum, normalize) runs on device.
"""

import os as _os
import sys

import numpy as np

try:
    import concourse  # noqa: F401
except ImportError:  # pragma: no cover
    sys.path.insert(0, "/opt/trn_rl_repo")

from contextlib import ExitStack

import concourse.mybir as mybir
from concourse import bacc, bass_utils, tile

N_NODES = 50000
N_EDGES = 1_600_000
D = 256
N_CORES = 8
SPLIT = 32768       # int16 gather-index window
NSA = 33            # slots covering the core's low-node slice
NSB = 18            # slots covering the core's high-node slice
NS = NSA + NSB

_PROGRAM_CACHE: dict = {}
LAST_NC = None  # exposed for test harness introspection (TimelineSim)

MAX_GATHER = 1024   # SWDGE descriptor ring capacity per dma_gather call
RING_L = 96         # L-stream SBUF ring, in 128-row tiles (multiple of 8)
RING_H = 56         # H-stream ring


def _pad_calls(rows):
    return -(-rows // MAX_GATHER) * MAX_GATHER


def _build_program(spans):
    """Build + compile the (uniform-across-cores, SPMD) per-core program.

    spans = (aL, bL, aH, bH): per-slot stream-tile intervals, the union
    over the 8 cores of each slot's L/H stream coverage. The program
    matmuls every (slot, tile) pair in these intervals; per-core srel
    data masks which rows of the tile actually belong to the slot.
    """
    aL, bL, aH, bH = (list(v) for v in spans)
    nt_l, nt_h = max(bL), max(bH)
    rows_l, rows_h = _pad_calls(nt_l * 128), _pad_calls(nt_h * 128)
    ni16 = (rows_l + rows_h) // 16
    ncol = sum(b - a for a, b in zip(aL, bL)) + sum(
        b - a for a, b in zip(aH, bH)
    )

    nc = bacc.Bacc(
        "TRN2", target_bir_lowering=False, debug=False, num_devices=N_CORES,
    )

    feat_d = nc.dram_tensor(
        "featb", (N_NODES, D), mybir.dt.bfloat16, kind="ExternalInput"
    ).ap()
    gidx_d = nc.dram_tensor(
        "gidx", (128, ni16), mybir.dt.int16, kind="ExternalInput"
    ).ap()
    srel_d = nc.dram_tensor(
        "srel", (128, ncol), mybir.dt.bfloat16, kind="ExternalInput"
    ).ap()
    cnt1_d = nc.dram_tensor(
        "cnt1", (128, NS), mybir.dt.float32, kind="ExternalInput"
    ).ap()
    out_d = nc.dram_tensor(
        "out", (NS * 128, D), mybir.dt.bfloat16, kind="ExternalOutput"
    ).ap()

    feat_lo = feat_d[0:SPLIT, :]
    feat_hi = feat_d[SPLIT:N_NODES, :]

    with tile.TileContext(nc) as tc:
        with ExitStack() as ctx:
            ob = int(_os.environ.get("OH_BUFS", "4"))
            fb = int(_os.environ.get("FIN_BUFS", "3"))
            pb = int(_os.environ.get("PSUM_BUFS", "4"))
            const_pool = ctx.enter_context(tc.tile_pool(name="const", bufs=1))
            oh_pool = ctx.enter_context(tc.tile_pool(name="oh", bufs=ob))
            fin_pool = ctx.enter_context(tc.tile_pool(name="fin", bufs=fb))
            psum_pool = ctx.enter_context(
                tc.tile_pool(name="psum", bufs=pb, space="PSUM")
            )

            # persistent metadata in SBUF (gidx loaded in chunks so early
            # gathers don't wait on the full index transfer)
            gidx_sb = const_pool.tile([128, ni16], mybir.dt.int16)
            n_ld = 8
            ld_bounds = [ni16 * i // n_ld for i in range(n_ld + 1)]
            for a, b in zip(ld_bounds[:-1], ld_bounds[1:]):
                if b > a:
                    nc.sync.dma_start(gidx_sb[:, a:b], gidx_d[:, a:b])
            # srel rides the wire as bf16 (values are small integers, exact)
            # and is widened on device: tensor_scalar's scalar operand must
            # be f32.
            srel_bf = const_pool.tile([128, ncol], mybir.dt.bfloat16)
            nc.sync.dma_start(srel_bf[:], srel_d[:])
            srel_sb = const_pool.tile([128, ncol], mybir.dt.float32)
            nc.vector.tensor_copy(srel_sb[:], srel_bf[:])
            cnt1_sb = const_pool.tile([128, NS], mybir.dt.float32)
            nc.sync.dma_start(cnt1_sb[:], cnt1_d[:])

            iota_i = const_pool.tile([128, 128], mybir.dt.int32)
            nc.gpsimd.iota(iota_i[:], pattern=[[1, 128]], base=0, channel_multiplier=0)
            iota_f = const_pool.tile([128, 128], mybir.dt.bfloat16)
            nc.vector.tensor_copy(iota_f[:], iota_i[:])

            ring_l = const_pool.tile([128, RING_L, D], mybir.dt.bfloat16)
            ring_h = const_pool.tile([128, RING_H, D], mybir.dt.bfloat16)

            def emit_call(ring, ring_sz, src, row0, col0):
                """One full 1024-row gather call of the given stream."""
                s0 = (row0 // 128) % ring_sz
                nc.gpsimd.dma_gather(
                    ring[:, s0 : s0 + MAX_GATHER // 128, :], src,
                    gidx_sb[:, col0 + row0 // 16 : col0 + (row0 + MAX_GATHER) // 16],
                    num_idxs=MAX_GATHER, num_idxs_reg=MAX_GATHER,
                    elem_size=D, elem_step=D,
                )
                return row0 + MAX_GATHER

            done_l = 0  # stream rows gathered so far
            done_h = 0
            col = 0     # srel column cursor (host layout matches this order)
            for g in range(NS):
                while done_l < bL[g] * 128:
                    done_l = emit_call(ring_l, RING_L, feat_lo, done_l, 0)
                while done_h < bH[g] * 128:
                    done_h = emit_call(ring_h, RING_H, feat_hi, done_h,
                                       rows_l // 16)

                n_mm = (bL[g] - aL[g]) + (bH[g] - aH[g])
                psum = psum_pool.tile([128, D], mybir.dt.float32, tag="ps")
                k = 0
                for m in range(aL[g], bL[g]):
                    oh = oh_pool.tile([128, 128], mybir.dt.bfloat16, tag="oh")
                    nc.vector.tensor_scalar(
                        oh[:], iota_f[:], srel_sb[:, col : col + 1], None,
                        op0=mybir.AluOpType.is_equal,
                    )
                    k += 1
                    nc.tensor.matmul(
                        psum[:], oh[:], ring_l[:, m % RING_L, :],
                        start=(k == 1), stop=(k == n_mm),
                    )
                    col += 1
                for m in range(aH[g], bH[g]):
                    oh = oh_pool.tile([128, 128], mybir.dt.bfloat16, tag="oh")
                    nc.vector.tensor_scalar(
                        oh[:], iota_f[:], srel_sb[:, col : col + 1], None,
                        op0=mybir.AluOpType.is_equal,
                    )
                    k += 1
                    nc.tensor.matmul(
                        psum[:], oh[:], ring_h[:, m % RING_H, :],
                        start=(k == 1), stop=(k == n_mm),
                    )
                    col += 1

                rec = fin_pool.tile([128, 1], mybir.dt.float32, tag="rec")
                nc.vector.reciprocal(rec[:], cnt1_sb[:, g : g + 1])
                o_sb = fin_pool.tile([128, D], mybir.dt.bfloat16, tag="o")
                nc.vector.tensor_scalar_mul(o_sb[:], psum[:], rec[:])
                nc.sync.dma_start(out_d[g * 128 : (g + 1) * 128, :], o_sb[:])

    nc.compile()
    return nc


def _preprocess(features, neighbor_idx, segment_ids):
    """Host-side shard/index metadata construction (integers only)."""
    feat = np.ascontiguousarray(np.asarray(features, dtype=np.float32))
    seg = np.asarray(segment_ids).astype(np.int64)
    nid = np.asarray(neighbor_idx).astype(np.int64)
    n_edges = seg.shape[0]

    bf16 = mybir.dt.np(mybir.dt.bfloat16)
    featb = feat.astype(bf16)
    deg = np.bincount(seg, minlength=N_NODES)

    # two-range node sharding: per-core slices of the low and high dest
    # nodes, edge-balanced within each class
    e_low = int(np.searchsorted(seg, SPLIT))
    lowb = [0]
    for c in range(1, N_CORES):
        lowb.append(int(seg[min(c * e_low // N_CORES, max(e_low - 1, 0))]))
    lowb.append(SPLIT)
    highb = [SPLIT]
    for c in range(1, N_CORES):
        highb.append(
            int(seg[min(e_low + c * (n_edges - e_low) // N_CORES, n_edges - 1)])
        )
    highb.append(N_NODES)

    # per-core merged (regular + self-loop) edge lists in dest order, and
    # per-node class-split prefix sums; dest ids are core-relative with the
    # high slice appended after the low slice
    cores = []
    for c in range(N_CORES):
        nn_a = lowb[c + 1] - lowb[c]
        segs, xs = [], []
        for b0, b1, off in (
            (lowb[c], lowb[c + 1], 0),
            (highb[c], highb[c + 1], nn_a),
        ):
            lo, hi = np.searchsorted(seg, [b0, b1])
            nn = b1 - b0
            segs.append(
                np.concatenate([seg[lo:hi] - np.int64(b0), np.arange(nn)]) + off
            )
            xs.append(np.concatenate([nid[lo:hi], np.arange(b0, b1)]))
        s = np.concatenate(segs)
        x = np.concatenate(xs)
        order = np.argsort(s, kind="stable")
        s, x = s[order], x[order]
        nn = nn_a + (highb[c + 1] - highb[c])
        is_l = x < SPLIT
        cum_l = np.concatenate([[0], np.cumsum(np.bincount(s[is_l], minlength=nn))])
        cum_h = np.concatenate([[0], np.cumsum(np.bincount(s[~is_l], minlength=nn))])
        cores.append((s, x, nn_a, nn, cum_l, cum_h))

    # anchored slot packing: per core, choose <=128-node slot boundaries
    # tracking shared cumulative L/H stream targets so every core's slot-g
    # stream interval lands in (nearly) the same tiles
    node_bnds_all = []
    st_l = np.zeros((N_CORES, NS), np.int64)
    en_l = np.zeros((N_CORES, NS), np.int64)
    st_h = np.zeros((N_CORES, NS), np.int64)
    en_h = np.zeros((N_CORES, NS), np.int64)
    for c, (s, x, nn_a, nn, cum_l, cum_h) in enumerate(cores):
        node_bnds = [0]
        i = 0
        for g in range(NS):
            if g == NSA - 1:
                j = nn_a
            elif g == NS - 1:
                j = nn
            else:
                l_mid, h_mid = cum_l[nn_a], cum_h[nn_a]
                if g < NSA:
                    t_l = l_mid * (g + 1) / NSA
                    t_h = h_mid * (g + 1) / NSA
                    part_end = nn_a
                else:
                    t_l = l_mid + (cum_l[nn] - l_mid) * (g + 1 - NSA) / NSB
                    t_h = h_mid + (cum_h[nn] - h_mid) * (g + 1 - NSA) / NSB
                    part_end = nn
                js = np.arange(i + 1, min(i + 128, part_end) + 1)
                cost = np.abs(cum_l[js] - t_l) + np.abs(cum_h[js] - t_h)
                j = int(js[np.argmin(cost)])
            assert j - i <= 128
            st_l[c, g], en_l[c, g] = cum_l[i], cum_l[j]
            st_h[c, g], en_h[c, g] = cum_h[i], cum_h[j]
            node_bnds.append(j)
            i = j
        node_bnds_all.append(node_bnds)

    aL = (st_l.min(0) // 128).tolist()
    bL = (-(-en_l.max(0) // 128)).tolist()
    aH = (st_h.min(0) // 128).tolist()
    bH = (-(-en_h.max(0) // 128)).tolist()
    spans = (tuple(aL), tuple(bL), tuple(aH), tuple(bH))
    nt_l, nt_h = max(bL), max(bH)
    rows_l, rows_h = _pad_calls(nt_l * 128), _pad_calls(nt_h * 128)
    ncol = sum(b - a for a, b in zip(aL, bL)) + sum(
        b - a for a, b in zip(aH, bH)
    )

    in_maps = []
    slot_maps = []
    for c, (s, x, nn_a, nn, cum_l, cum_h) in enumerate(cores):
        node_bnds = node_bnds_all[c]
        is_l = x < SPLIT
        # dense class streams in dest order; within each slot's run, sort
        # by source row for HBM locality (order within a slot is free)
        xl, sl_ = x[is_l], s[is_l]
        xh, sh_ = x[~is_l] - SPLIT, s[~is_l]
        for g in range(NS):
            i, j = node_bnds[g], node_bnds[g + 1]
            for xs_, ss_, cum in ((xl, sl_, cum_l), (xh, sh_, cum_h)):
                a, b = int(cum[i]), int(cum[j])
                o = np.argsort(xs_[a:b], kind="stable")
                xs_[a:b], ss_[a:b] = xs_[a:b][o], ss_[a:b][o]

        gidx_all = np.zeros(rows_l + rows_h, np.int16)
        gidx_all[: len(xl)] = xl.astype(np.int16)
        gidx_all[rows_l : rows_l + len(xh)] = xh.astype(np.int16)

        srel_all = np.full((ncol, 128), -1.0, np.float32)
        cnt1 = np.ones((128, NS), np.float32)
        col = 0
        for g in range(NS):
            i, j = node_bnds[g], node_bnds[g + 1]
            for (a_t, b_t, st, en, ss_) in (
                (aL[g], bL[g], int(cum_l[i]), int(cum_l[j]), sl_),
                (aH[g], bH[g], int(cum_h[i]), int(cum_h[j]), sh_),
            ):
                for m in range(a_t, b_t):
                    r0, r1 = max(128 * m, st), min(128 * m + 128, en)
                    if r1 > r0:
                        srel_all[col, r0 - 128 * m : r1 - 128 * m] = (
                            ss_[r0:r1] - i
                        )
                    col += 1
            width = j - i
            if width:
                if i < nn_a:
                    abs_base = lowb[c] + i
                else:
                    abs_base = highb[c] + (i - nn_a)
                cnt1[:width, g] = 1.0 + deg[abs_base : abs_base + width]
        assert col == ncol

        gidx_w = np.ascontiguousarray(
            np.tile(gidx_all.reshape(-1, 16).T, (8, 1))
        )
        in_maps.append(
            {
                "featb": featb,
                "gidx": gidx_w,
                "srel": np.ascontiguousarray(srel_all.T).astype(bf16),
                "cnt1": cnt1,
            }
        )
        sm = []
        for g in range(NS):
            i, j = node_bnds[g], node_bnds[g + 1]
            if i < nn_a:
                sm.append((lowb[c] + i, j - i))
            else:
                sm.append((highb[c] + (i - nn_a), j - i))
        slot_maps.append(sm)
    return spans, in_maps, slot_maps


def kernel(features, neighbor_idx, segment_ids):
    global LAST_NC
    spans, in_maps, slot_maps = _preprocess(
        features, neighbor_idx, segment_ids
    )

    if spans not in _PROGRAM_CACHE:
        _PROGRAM_CACHE[spans] = _build_program(spans)
    nc = _PROGRAM_CACHE[spans]
    LAST_NC = nc

    try:
        res = bass_utils.run_bass_kernel_spmd(
            nc, in_maps, core_ids=list(range(N_CORES))
        )
    except Exception:
        # transient axon/device hiccups (e.g. recovering from a prior wedge)
        # have been observed to clear after a short pause
        import time

        time.sleep(20)
        res = bass_utils.run_bass_kernel_spmd(
            nc, in_maps, core_ids=list(range(N_CORES))
        )

    out = np.empty((N_NODES, D), np.float32)
    for c in range(N_CORES):
        oc = res.results[c]["out"].astype(np.float32)
        for g, (abs_base, width) in enumerate(slot_maps[c]):
            if width:
                out[abs_base : abs_base + width] = oc[g * 128 : g * 128 + width]
    return out


# revision 24
# speedup vs baseline: 1.0386x; 1.0096x over previous
"""GCNAggregator Trainium2 Bass kernel.

out[i] = (sum_{e: seg[e]==i} features[neighbor_idx[e]] + features[i]) / (deg_i + 1)

Strategy (8 NeuronCores, SPMD):
  - dma_gather indices are int16 (max 32767) but the table has 50000 rows,
    so every gathered row is classed L (row < 32768, gathered from the
    table base) or H (row >= 32768, gathered from an offset view). Each
    core is given a contiguous slice of the LOW dest nodes and a
    contiguous slice of the HIGH dest nodes, edge-balanced within each
    class, so per-core L/H gather totals match across cores to ~0.1%
    (one self-loop edge per node is folded in, and self rows are all-L on
    low nodes / all-H on high nodes).
  - Features ride as a bf16 table (512B/row -- the cost model's DMA
    sweet spot). Rel err ~3e-3 end-to-end, well inside the 2e-2 gate.
  - Per core, dest nodes are packed into 51 "slots" of <=128 consecutive
    nodes. The L rows of all slots form one dense stream in dest order
    (H likewise): no per-slot alignment padding. Slot boundaries are
    anchored to shared cumulative targets so every core's slot-g stream
    interval lands within a tile or two of the same place.
  - Streams are gathered with full 1024-descriptor dma_gather calls
    (the SWDGE ring caps at 1024 descriptors per call; bigger rings wedge
    the device) into circular SBUF rings of 128-row tiles, ~1us fixed
    Pool-engine cost per call. Gather calls ignore slot boundaries.
  - Segment-sum per slot on the tensor engine: for each stream tile
    overlapping the slot's interval on ANY core,
        psum[128 nodes, 256] += onehot[128 rows, 128 nodes]^T @ ring[tile]
    with the bf16 one-hot built on device by is_equal(iota, srel) from
    per-(tile,slot) relative dest ids; rows of a boundary tile that
    belong to the neighboring slot carry srel -1 there and are picked up
    by that slot's own matmul over the same tile. PSUM accumulates fp32.
  - Finalize per slot: out = psum * 1/(deg+1) (bf16), DMA out.
  - Engine budget per core (TimelineSim): DMA ~314us (98% busy, the
    bottleneck: 206k gather descriptors x 512B at 360GB/s aggregate),
    Pool ~273us, PE ~215us, DVE ~212us.

The host only computes integer index metadata (shard boundaries, stream
index layouts, relative segment ids, degrees); all floating point work
(gather, segment sum, normalize) runs on device.
"""

import os as _os
import sys

import numpy as np

try:
    import concourse  # noqa: F401
except ImportError:  # pragma: no cover
    sys.path.insert(0, "/opt/trn_rl_repo")

from contextlib import ExitStack

import concourse.mybir as mybir
from concourse import bacc, bass_utils, tile

N_NODES = 50000
N_EDGES = 1_600_000
D = 256
N_CORES = 8
SPLIT = 32768       # int16 gather-index window
NSA = 33            # slots covering the core's low-node slice
NSB = 18            # slots covering the core's high-node slice
NS = NSA + NSB

_PROGRAM_CACHE: dict = {}
LAST_NC = None  # exposed for test harness introspection (TimelineSim)

MAX_GATHER = 1024   # SWDGE descriptor ring capacity per dma_gather call
RING_L = 96         # L-stream SBUF ring, in 128-row tiles (multiple of 8)
RING_H = 56         # H-stream ring


def _pad_calls(rows):
    return -(-rows // MAX_GATHER) * MAX_GATHER


def _build_program(spans):
    """Build + compile the (uniform-across-cores, SPMD) per-core program.

    spans = (aL, bL, aH, bH): per-slot stream-tile intervals, the union
    over the 8 cores of each slot's L/H stream coverage. The program
    matmuls every (slot, tile) pair in these intervals; per-core srel
    data masks which rows of the tile actually belong to the slot.
    """
    aL, bL, aH, bH = (list(v) for v in spans)
    nt_l, nt_h = max(bL), max(bH)
    rows_l, rows_h = _pad_calls(nt_l * 128), _pad_calls(nt_h * 128)
    ni16 = (rows_l + rows_h) // 16
    ncol = sum(b - a for a, b in zip(aL, bL)) + sum(
        b - a for a, b in zip(aH, bH)
    )

    nc = bacc.Bacc(
        "TRN2", target_bir_lowering=False, debug=False, num_devices=N_CORES,
    )

    feat_d = nc.dram_tensor(
        "featb", (N_NODES, D), mybir.dt.bfloat16, kind="ExternalInput"
    ).ap()
    gidx_d = nc.dram_tensor(
        "gidxf", (16, ni16), mybir.dt.float32, kind="ExternalInput"
    ).ap()
    rep_d = nc.dram_tensor(
        "repmat", (16, 128), mybir.dt.float32, kind="ExternalInput"
    ).ap()
    srel_d = nc.dram_tensor(
        "srel", (128, ncol), mybir.dt.bfloat16, kind="ExternalInput"
    ).ap()
    cnt1_d = nc.dram_tensor(
        "cnt1", (128, NS), mybir.dt.float32, kind="ExternalInput"
    ).ap()
    out_d = nc.dram_tensor(
        "out", (NS * 128, D), mybir.dt.bfloat16, kind="ExternalOutput"
    ).ap()

    feat_lo = feat_d[0:SPLIT, :]
    feat_hi = feat_d[SPLIT:N_NODES, :]

    with tile.TileContext(nc) as tc:
        with ExitStack() as ctx:
            ob = int(_os.environ.get("OH_BUFS", "4"))
            fb = int(_os.environ.get("FIN_BUFS", "3"))
            pb = int(_os.environ.get("PSUM_BUFS", "4"))
            const_pool = ctx.enter_context(tc.tile_pool(name="const", bufs=1))
            oh_pool = ctx.enter_context(tc.tile_pool(name="oh", bufs=ob))
            fin_pool = ctx.enter_context(tc.tile_pool(name="fin", bufs=fb))
            psum_pool = ctx.enter_context(
                tc.tile_pool(name="psum", bufs=pb, space="PSUM")
            )

            # gather indices ride the wire once as [16, ni16] f32 (exact for
            # idx < 2^24) and are replicated to the 128-partition wrapped
            # layout the SWDGE ucode needs via one-hot f32 matmuls on the
            # (otherwise idle-at-start) tensor engine, with PSUM->SBUF int16
            # copies on the idle Activation engine. Chunks are produced
            # just-in-time between slot accumulation chains, ahead of the
            # gather calls that read them.
            gidx_sb = const_pool.tile([128, ni16], mybir.dt.int16)
            gidxf_sb = const_pool.tile([16, ni16], mybir.dt.float32)
            rep_sb = const_pool.tile([16, 128], mybir.dt.float32)
            nc.sync.dma_start(rep_sb[:], rep_d[:])
            n_ld = 8
            ld_bounds = [ni16 * i // n_ld for i in range(n_ld + 1)]
            for a, b in zip(ld_bounds[:-1], ld_bounds[1:]):
                if b > a:
                    nc.sync.dma_start(gidxf_sb[:, a:b], gidx_d[:, a:b])
            rep_pool = ctx.enter_context(
                tc.tile_pool(name="rpsum", bufs=2, space="PSUM")
            )
            # independent replication cursors for the L and H col regions
            rep_state = {"L": 0, "H": rows_l // 16}
            rep_end = {"L": rows_l // 16, "H": ni16}

            def rep_to(region, col_need):
                while rep_state[region] < min(col_need, rep_end[region]):
                    a = rep_state[region]
                    b = min(a + 512, rep_end[region])
                    rp = rep_pool.tile([128, 512], mybir.dt.float32, tag="rp")
                    nc.tensor.matmul(
                        rp[:, : b - a], rep_sb[:], gidxf_sb[:, a:b],
                        start=True, stop=True,
                    )
                    nc.scalar.copy(gidx_sb[:, a:b], rp[:, : b - a])
                    rep_state[region] = b
            # srel rides the wire as bf16 (values are small integers, exact)
            # and is widened on device: tensor_scalar's scalar operand must
            # be f32.
            srel_bf = const_pool.tile([128, ncol], mybir.dt.bfloat16)
            nc.sync.dma_start(srel_bf[:], srel_d[:])
            srel_sb = const_pool.tile([128, ncol], mybir.dt.float32)
            nc.vector.tensor_copy(srel_sb[:], srel_bf[:])
            cnt1_sb = const_pool.tile([128, NS], mybir.dt.float32)
            nc.sync.dma_start(cnt1_sb[:], cnt1_d[:])

            iota_i = const_pool.tile([128, 128], mybir.dt.int32)
            nc.gpsimd.iota(iota_i[:], pattern=[[1, 128]], base=0, channel_multiplier=0)
            iota_f = const_pool.tile([128, 128], mybir.dt.bfloat16)
            nc.vector.tensor_copy(iota_f[:], iota_i[:])

            ring_l = const_pool.tile([128, RING_L, D], mybir.dt.bfloat16)
            ring_h = const_pool.tile([128, RING_H, D], mybir.dt.bfloat16)

            def emit_call(ring, ring_sz, src, row0, col0):
                """One full 1024-row gather call of the given stream."""
                s0 = (row0 // 128) % ring_sz
                nc.gpsimd.dma_gather(
                    ring[:, s0 : s0 + MAX_GATHER // 128, :], src,
                    gidx_sb[:, col0 + row0 // 16 : col0 + (row0 + MAX_GATHER) // 16],
                    num_idxs=MAX_GATHER, num_idxs_reg=MAX_GATHER,
                    elem_size=D, elem_step=D,
                )
                return row0 + MAX_GATHER

            done_l = 0  # stream rows gathered so far
            done_h = 0
            col = 0     # srel column cursor (host layout matches this order)
            for g in range(NS):
                # replicate the idx cols this slot's gather calls will read,
                # plus one chunk of lookahead (outside any psum matmul chain)
                rep_to("L", _pad_calls(bL[g] * 128) // 16 + 512)
                rep_to("H", rows_l // 16 + _pad_calls(bH[g] * 128) // 16 + 512)
                while done_l < bL[g] * 128:
                    done_l = emit_call(ring_l, RING_L, feat_lo, done_l, 0)
                while done_h < bH[g] * 128:
                    done_h = emit_call(ring_h, RING_H, feat_hi, done_h,
                                       rows_l // 16)

                n_mm = (bL[g] - aL[g]) + (bH[g] - aH[g])
                psum = psum_pool.tile([128, D], mybir.dt.float32, tag="ps")
                k = 0
                for m in range(aL[g], bL[g]):
                    oh = oh_pool.tile([128, 128], mybir.dt.bfloat16, tag="oh")
                    nc.vector.tensor_scalar(
                        oh[:], iota_f[:], srel_sb[:, col : col + 1], None,
                        op0=mybir.AluOpType.is_equal,
                    )
                    k += 1
                    nc.tensor.matmul(
                        psum[:], oh[:], ring_l[:, m % RING_L, :],
                        start=(k == 1), stop=(k == n_mm),
                    )
                    col += 1
                for m in range(aH[g], bH[g]):
                    oh = oh_pool.tile([128, 128], mybir.dt.bfloat16, tag="oh")
                    nc.vector.tensor_scalar(
                        oh[:], iota_f[:], srel_sb[:, col : col + 1], None,
                        op0=mybir.AluOpType.is_equal,
                    )
                    k += 1
                    nc.tensor.matmul(
                        psum[:], oh[:], ring_h[:, m % RING_H, :],
                        start=(k == 1), stop=(k == n_mm),
                    )
                    col += 1

                rec = fin_pool.tile([128, 1], mybir.dt.float32, tag="rec")
                nc.vector.reciprocal(rec[:], cnt1_sb[:, g : g + 1])
                o_sb = fin_pool.tile([128, D], mybir.dt.bfloat16, tag="o")
                nc.vector.tensor_scalar_mul(o_sb[:], psum[:], rec[:])
                nc.sync.dma_start(out_d[g * 128 : (g + 1) * 128, :], o_sb[:])

    nc.compile()
    return nc


def _preprocess(features, neighbor_idx, segment_ids):
    """Host-side shard/index metadata construction (integers only)."""
    feat = np.ascontiguousarray(np.asarray(features, dtype=np.float32))
    seg = np.asarray(segment_ids).astype(np.int64)
    nid = np.asarray(neighbor_idx).astype(np.int64)
    n_edges = seg.shape[0]

    bf16 = mybir.dt.np(mybir.dt.bfloat16)
    featb = feat.astype(bf16)
    deg = np.bincount(seg, minlength=N_NODES)

    # two-range node sharding: per-core slices of the low and high dest
    # nodes, edge-balanced within each class
    e_low = int(np.searchsorted(seg, SPLIT))
    lowb = [0]
    for c in range(1, N_CORES):
        lowb.append(int(seg[min(c * e_low // N_CORES, max(e_low - 1, 0))]))
    lowb.append(SPLIT)
    highb = [SPLIT]
    for c in range(1, N_CORES):
        highb.append(
            int(seg[min(e_low + c * (n_edges - e_low) // N_CORES, n_edges - 1)])
        )
    highb.append(N_NODES)

    # per-core merged (regular + self-loop) edge lists in dest order, and
    # per-node class-split prefix sums; dest ids are core-relative with the
    # high slice appended after the low slice
    cores = []
    for c in range(N_CORES):
        nn_a = lowb[c + 1] - lowb[c]
        segs, xs = [], []
        for b0, b1, off in (
            (lowb[c], lowb[c + 1], 0),
            (highb[c], highb[c + 1], nn_a),
        ):
            lo, hi = np.searchsorted(seg, [b0, b1])
            nn = b1 - b0
            segs.append(
                np.concatenate([seg[lo:hi] - np.int64(b0), np.arange(nn)]) + off
            )
            xs.append(np.concatenate([nid[lo:hi], np.arange(b0, b1)]))
        s = np.concatenate(segs)
        x = np.concatenate(xs)
        order = np.argsort(s, kind="stable")
        s, x = s[order], x[order]
        nn = nn_a + (highb[c + 1] - highb[c])
        is_l = x < SPLIT
        cum_l = np.concatenate([[0], np.cumsum(np.bincount(s[is_l], minlength=nn))])
        cum_h = np.concatenate([[0], np.cumsum(np.bincount(s[~is_l], minlength=nn))])
        cores.append((s, x, nn_a, nn, cum_l, cum_h))

    # anchored slot packing: per core, choose <=128-node slot boundaries
    # tracking shared cumulative L/H stream targets so every core's slot-g
    # stream interval lands in (nearly) the same tiles
    node_bnds_all = []
    st_l = np.zeros((N_CORES, NS), np.int64)
    en_l = np.zeros((N_CORES, NS), np.int64)
    st_h = np.zeros((N_CORES, NS), np.int64)
    en_h = np.zeros((N_CORES, NS), np.int64)
    for c, (s, x, nn_a, nn, cum_l, cum_h) in enumerate(cores):
        node_bnds = [0]
        i = 0
        for g in range(NS):
            if g == NSA - 1:
                j = nn_a
            elif g == NS - 1:
                j = nn
            else:
                l_mid, h_mid = cum_l[nn_a], cum_h[nn_a]
                if g < NSA:
                    t_l = l_mid * (g + 1) / NSA
                    t_h = h_mid * (g + 1) / NSA
                    part_end = nn_a
                else:
                    t_l = l_mid + (cum_l[nn] - l_mid) * (g + 1 - NSA) / NSB
                    t_h = h_mid + (cum_h[nn] - h_mid) * (g + 1 - NSA) / NSB
                    part_end = nn
                js = np.arange(i + 1, min(i + 128, part_end) + 1)
                cost = np.abs(cum_l[js] - t_l) + np.abs(cum_h[js] - t_h)
                j = int(js[np.argmin(cost)])
            assert j - i <= 128
            st_l[c, g], en_l[c, g] = cum_l[i], cum_l[j]
            st_h[c, g], en_h[c, g] = cum_h[i], cum_h[j]
            node_bnds.append(j)
            i = j
        node_bnds_all.append(node_bnds)

    aL = (st_l.min(0) // 128).tolist()
    bL = (-(-en_l.max(0) // 128)).tolist()
    aH = (st_h.min(0) // 128).tolist()
    bH = (-(-en_h.max(0) // 128)).tolist()
    spans = (tuple(aL), tuple(bL), tuple(aH), tuple(bH))
    nt_l, nt_h = max(bL), max(bH)
    rows_l, rows_h = _pad_calls(nt_l * 128), _pad_calls(nt_h * 128)
    ncol = sum(b - a for a, b in zip(aL, bL)) + sum(
        b - a for a, b in zip(aH, bH)
    )

    in_maps = []
    slot_maps = []
    for c, (s, x, nn_a, nn, cum_l, cum_h) in enumerate(cores):
        node_bnds = node_bnds_all[c]
        is_l = x < SPLIT
        # dense class streams in dest order; within each slot's run, sort
        # by source row for HBM locality (order within a slot is free)
        xl, sl_ = x[is_l], s[is_l]
        xh, sh_ = x[~is_l] - SPLIT, s[~is_l]
        for g in range(NS):
            i, j = node_bnds[g], node_bnds[g + 1]
            for xs_, ss_, cum in ((xl, sl_, cum_l), (xh, sh_, cum_h)):
                a, b = int(cum[i]), int(cum[j])
                o = np.argsort(xs_[a:b], kind="stable")
                xs_[a:b], ss_[a:b] = xs_[a:b][o], ss_[a:b][o]

        gidx_all = np.zeros(rows_l + rows_h, np.int16)
        gidx_all[: len(xl)] = xl.astype(np.int16)
        gidx_all[rows_l : rows_l + len(xh)] = xh.astype(np.int16)

        srel_all = np.full((ncol, 128), -1.0, np.float32)
        cnt1 = np.ones((128, NS), np.float32)
        col = 0
        for g in range(NS):
            i, j = node_bnds[g], node_bnds[g + 1]
            for (a_t, b_t, st, en, ss_) in (
                (aL[g], bL[g], int(cum_l[i]), int(cum_l[j]), sl_),
                (aH[g], bH[g], int(cum_h[i]), int(cum_h[j]), sh_),
            ):
                for m in range(a_t, b_t):
                    r0, r1 = max(128 * m, st), min(128 * m + 128, en)
                    if r1 > r0:
                        srel_all[col, r0 - 128 * m : r1 - 128 * m] = (
                            ss_[r0:r1] - i
                        )
                    col += 1
            width = j - i
            if width:
                if i < nn_a:
                    abs_base = lowb[c] + i
                else:
                    abs_base = highb[c] + (i - nn_a)
                cnt1[:width, g] = 1.0 + deg[abs_base : abs_base + width]
        assert col == ncol

        gidx_w = np.ascontiguousarray(
            gidx_all.reshape(-1, 16).T.astype(np.float32)
        )
        in_maps.append(
            {
                "featb": featb,
                "gidxf": gidx_w,
                "repmat": np.ascontiguousarray(
                    (np.arange(128)[None, :] % 16 == np.arange(16)[:, None])
                    .astype(np.float32)
                ),
                "srel": np.ascontiguousarray(srel_all.T).astype(bf16),
                "cnt1": cnt1,
            }
        )
        sm = []
        for g in range(NS):
            i, j = node_bnds[g], node_bnds[g + 1]
            if i < nn_a:
                sm.append((lowb[c] + i, j - i))
            else:
                sm.append((highb[c] + (i - nn_a), j - i))
        slot_maps.append(sm)
    return spans, in_maps, slot_maps


def kernel(features, neighbor_idx, segment_ids):
    global LAST_NC
    spans, in_maps, slot_maps = _preprocess(
        features, neighbor_idx, segment_ids
    )

    if spans not in _PROGRAM_CACHE:
        _PROGRAM_CACHE[spans] = _build_program(spans)
    nc = _PROGRAM_CACHE[spans]
    LAST_NC = nc

    try:
        res = bass_utils.run_bass_kernel_spmd(
            nc, in_maps, core_ids=list(range(N_CORES))
        )
    except Exception:
        # transient axon/device hiccups (e.g. recovering from a prior wedge)
        # have been observed to clear after a short pause
        import time

        time.sleep(20)
        res = bass_utils.run_bass_kernel_spmd(
            nc, in_maps, core_ids=list(range(N_CORES))
        )

    out = np.empty((N_NODES, D), np.float32)
    for c in range(N_CORES):
        oc = res.results[c]["out"].astype(np.float32)
        for g, (abs_base, width) in enumerate(slot_maps[c]):
            if width:
                out[abs_base : abs_base + width] = oc[g * 128 : g * 128 + width]
    return out
